# revision 56
# baseline (speedup 1.0000x reference)
"""Cosine cross-attention (B=4, L=2048, D=1024, H=16, dh=64, tau=0.07) on 8 trn2 cores.

Sharding: core = b*2 + g  (b in 0..3 data-parallel, g in 0..1 head-group of 8 heads).

Engine-overlap-oriented structure:
  prologue: V-proj, K-proj (per-block norms+applies software-pipelined one
  block behind the projections), Q block 0. Q blocks 1..3, the out-projection
  chunks, and their norm work stream INTO the attention loop as deferred
  items (one popped per lk iteration) so the ACT exp pipeline never waits.

  - One ACT table for everything: softmax exp plus the norm factors
    1/||x|| = exp(-0.5*ln(ss + eps)) (ln(1/tau) folded into k's exponent
    bias). The streamed q-norm Ln/Exp pair is batched over columns 512:2048
    so the Ln<->Exp table switch is paid once. Squares are done on DVE.
  - Norm row-sums: accumulated row-select matmuls (seln) write all 128 psum
    partitions (zeros elsewhere), so the broadcast source has no junk rows.
  - Z: DVE reciprocal of the PSUM ones-row, GpSimd partition_broadcast,
    fused multiply into mt during the PSUM->SBUF move. No DMA, no PE.
  - Attention is lq-outer / m-inner; out-projection chunks for lq run inside
    the (lq+1, m=0) group's exp stream via the deferred-work queue, using
    the PV-accumulator pool so the score double-buffer is untouched.
  - Bulk x loads are big SWDGE transfers on SP (first xv block leads the
    queue); weights + output stores are issued from GpSimd.
  - bf16 storage for qt/kt/vg/et/mt/wo; f32 PSUM accumulation everywhere.
"""

import os

# some harnesses pin jax to cpu for the reference; this kernel needs the
# axon/neuron backend, so clear the pin before jax is first imported
if os.environ.get("JAX_PLATFORMS") == "cpu":
    del os.environ["JAX_PLATFORMS"]

import math

import numpy as np

import concourse.bacc as bacc
import concourse.tile as tile
from concourse import mybir
from concourse.bass_utils import run_bass_kernel_spmd

P = 128
L = 2048
D = 1024
DO = 512  # per-core output dims of q/k/v projections (8 heads * 64)
TAU = 0.07
NLB = L // 512   # 4 blocks of 512 along L
NLK = L // 128   # 16 chunks of 128 along L (keys)
NM = DO // P     # 4 dout chunks (head pairs)
NKC = D // P     # 8 contraction chunks for projections

F32 = mybir.dt.float32
F32R = mybir.dt.float32r
BF16 = mybir.dt.bfloat16
EXP = mybir.ActivationFunctionType.Exp
LN = mybir.ActivationFunctionType.Ln
MULT = mybir.AluOpType.mult
LNTAUINV = -math.log(TAU)  # fold 1/tau into k's norm factor

_CACHE = {}


def _emit(nc, prm, repeat=1, phases="pcd"):
    with tile.TileContext(nc) as tc:
        if repeat > 1:
            with tc.For_i(0, repeat, 1):
                _emit_body(nc, tc, prm, phases)
        else:
            _emit_body(nc, tc, prm, phases)


def _emit_body(nc, tc, prm, phases="pcd"):
    from contextlib import ExitStack
    with ExitStack() as stack:
        const = stack.enter_context(tc.tile_pool(name="const", bufs=1))
        persist = stack.enter_context(tc.tile_pool(name="persist", bufs=1))

        # ---- first x block loads lead the SP queue (v-proj gates on them) ----
        xp = stack.enter_context(tc.tile_pool(name="xp", bufs=3))

        def load_x(name, lb, eng=None):
            eng = eng or nc.sync
            sl = slice(lb * 512, (lb + 1) * 512)
            x0 = xp.tile([P, NKC // 2, 512], F32R, tag="x", name="x0")
            eng.dma_start(
                out=x0[:],
                in_=prm[name][0:512, sl].rearrange("(c p) i -> p c i", c=4))
            x1 = xp.tile([P, NKC // 2, 512], F32R, tag="x", name="x1")
            eng.dma_start(
                out=x1[:],
                in_=prm[name][512:1024, sl].rearrange("(c p) i -> p c i", c=4))
            return x0, x1

        xv_first = load_x("xv", 0)

        # ---- constants ----
        seln = const.tile([P, NM, P], F32R, tag="seln")
        nc.sync.dma_start(out=seln[:], in_=prm["seln"][:])
        selbc = const.tile([P, NM, P], F32R, tag="selbc")
        nc.sync.dma_start(out=selbc[:], in_=prm["selbc"][:])
        bq_t = const.tile([P, NM], F32, tag="bq")
        nc.sync.dma_start(out=bq_t[:], in_=prm["bqp"][:])
        bk_t = const.tile([P, NM], F32, tag="bk")
        nc.sync.dma_start(out=bk_t[:], in_=prm["bkp"][:])
        # col 0: ln-eps bias, col 1: ln(1/tau) (k's exponent bias)
        actc = const.tile([P, 2], F32, tag="actc")
        nc.sync.dma_start(out=actc[:], in_=prm["actc"][:])
        ones8 = const.tile([P, 8], BF16, tag="ones8")
        nc.sync.dma_start(out=ones8[:], in_=prm["ones8"][:])

        # ---- persistent tensors ----
        qt = [persist.tile([P, L], BF16, tag=f"qt{m}", name=f"qt{m}") for m in range(NM)]
        kt = [persist.tile([P, L], BF16, tag=f"kt{m}", name=f"kt{m}") for m in range(NM)]
        mt = [persist.tile([P, L], BF16, tag=f"mt{m}", name=f"mt{m}") for m in range(NM)]
        vg_all = persist.tile([P, NLK, 8, 65], BF16, tag="vg_all")
        nsq = {"q": persist.tile([P, L], F32R, tag="nsq_q", name="nsq_q"),
               "k": persist.tile([P, L], F32R, tag="nsq_k", name="nsq_k")}
        wot = persist.tile([P, NM, D], BF16, tag="wot")

        wp = stack.enter_context(tc.tile_pool(name="wp", bufs=2))
        sqp = stack.enter_context(tc.tile_pool(name="sqp", bufs=8))

        def load_w(name):
            w_t = wp.tile([P, NKC, DO], F32R, tag="w", name=f"w_{name}")
            nc.gpsimd.dma_start(
                out=w_t[:],
                in_=prm[name][:].rearrange("(c p) i -> p c i", c=NKC))
            return w_t

        wv_t = load_w("wv")
        wk_t = load_w("wk")

        # ---------------- V projection (natural layout) ----------------
        with tc.tile_pool(name="psV", bufs=2, space="PSUM") as psV:
            for lb in range(NLB if "p" in phases else 0):
                x0, x1 = xv_first if lb == 0 else load_x("xv", lb)
                for j in range(4):
                    pav = psV.tile([P, 512], F32, tag="pav")
                    for c8 in range(NKC):
                        xt = (x0 if c8 < 4 else x1)
                        nc.tensor.matmul(
                            pav[:],
                            lhsT=xt[:, c8 % 4, j * P:(j + 1) * P],
                            rhs=wv_t[:, c8, :],
                            start=(c8 == 0), stop=(c8 == NKC - 1))
                    lc = lb * 4 + j
                    nc.vector.tensor_copy(
                        out=vg_all[:, lc, :, 0:64],
                        in_=pav[:].rearrange("p (h d) -> p h d", h=8))
                    nc.vector.tensor_copy(out=vg_all[:, lc, :, 64],
                                          in_=ones8[:])
        wq_t = load_w("wq")  # reuses wv's slot; hidden behind k-projection
        nc.gpsimd.dma_start(
            out=wot[:],
            in_=prm["wo"][:].rearrange("(c p) i -> p c i", c=NM))

        # ---------------- Q/K projections + norm factors ----------------
        # Emission primitives shared by the pre-attention path (dedicated
        # psum banks, kc-outer, epilogues deferred one block so the PE never
        # waits on a DVE round trip) and the streamed-q path (small items
        # borrowing PV-pool slots inside the attention loop).
        def proj_mms_m(kind, lb, m, x0, x1, pa):
            w_t = wk_t if kind == "k" else wq_t
            for c8 in range(NKC):
                xt = (x0 if c8 < 4 else x1)
                nc.tensor.matmul(
                    pa[:], lhsT=w_t[:, c8, m * P:(m + 1) * P],
                    rhs=xt[:, c8 % 4, :],
                    start=(c8 == 0), stop=(c8 == NKC - 1))

        def bias_sq_m(kind, lb, m, pa):
            """bias-add into qt/kt (bf16/f32r) + squares; returns sq tile."""
            b_t = bq_t if kind == "q" else bk_t
            dst = qt if kind == "q" else kt
            blk = dst[m][:, slice(lb * 512, (lb + 1) * 512)]
            nc.vector.tensor_scalar_add(out=blk, in0=pa[:],
                                        scalar1=b_t[:, m:m + 1])
            sq_t = sqp.tile([P, 512], F32R, tag="sq")
            nc.vector.tensor_tensor(out=sq_t[:], in0=blk, in1=blk, op=MULT)
            return sq_t

        def nq_mms(kind, lb, ms, sqs, psn, rows=slice(0, P)):
            """Head-pair square-sums: accumulated row-select matmuls write ALL
            128 partitions of psn (zeros elsewhere -> no junk rows); only the
            `rows` half is copied into nsq (the other half may hold factors
            from an earlier pass)."""
            sl = slice(lb * 512, (lb + 1) * 512)
            for i, (m, sq_t) in enumerate(zip(ms, sqs)):
                nc.tensor.matmul(psn[:], lhsT=seln[:, m, :], rhs=sq_t[:],
                                 start=(i == 0), stop=(i == len(ms) - 1))
            nc.vector.tensor_copy(out=nsq[kind][rows, sl], in_=psn[rows, :])

        def emit_norms(kind, cols, rows=slice(0, P)):
            # 1/||x|| = exp(-0.5*ln(ss + eps)); ln(1/tau) folded into k's bias
            nb = nsq[kind][rows, cols]
            with nc.allow_low_precision(reason="norms via ln/exp"):
                nc.scalar.activation(out=nb, in_=nb, func=LN,
                                     bias=actc[rows, 0:1])
                if kind == "q":
                    nc.scalar.activation(out=nb, in_=nb, func=EXP, scale=-0.5)
                else:
                    nc.scalar.activation(out=nb, in_=nb, func=EXP, scale=-0.5,
                                         bias=actc[rows, 1:2])

        def emit_apply(kind, lb, m, bc):
            sl = slice(lb * 512, (lb + 1) * 512)
            nc.tensor.matmul(bc[:], lhsT=selbc[:, m, :],
                             rhs=nsq[kind][:, sl], start=True, stop=True)
            blk = (qt if kind == "q" else kt)[m][:, sl]
            nc.vector.tensor_tensor(out=blk, in0=blk, in1=bc[:], op=MULT)

        # pre-attention prologue: K's first two head-pairs (what the first
        # two attention groups read) + Q's first block. K's other head-pairs
        # and Q's other blocks stream into the attention loop as deferred
        # work. Software-pipelined: block i's norm/apply matmuls run behind
        # block i+1's projections so the PE never waits on a DVE round trip.
        LOW, HIGH = slice(0, 64), slice(64, P)
        with tc.tile_pool(name="psA", bufs=1, space="PSUM") as psA, \
             tc.tile_pool(name="psN", bufs=2, space="PSUM") as psN:
            sections = ([("k", lb, (0, 1, 2, 3)) for lb in range(NLB)]
                        + [("q", 0, (0, 1, 2, 3))] if "p" in phases else [])
            deferred = None
            for kind, lb, ms in sections + [(None, None, None)]:
                if kind is not None:
                    x0, x1 = load_x("x" + kind, lb)
                    pas = [psA.tile([P, 512], F32, tag=f"pa{m}", name="pa")
                           for m in ms]
                    for c8 in range(NKC):
                        xt = (x0 if c8 < 4 else x1)
                        for i, m in enumerate(ms):
                            nc.tensor.matmul(
                                pas[i][:],
                                lhsT=(wk_t if kind == "k" else wq_t)[
                                    :, c8, m * P:(m + 1) * P],
                                rhs=xt[:, c8 % 4, :],
                                start=(c8 == 0), stop=(c8 == NKC - 1))
                    sqs = [bias_sq_m(kind, lb, m, pas[i])
                           for i, m in enumerate(ms)]
                if deferred is not None:
                    dkind, dlb, dms, dsqs = deferred
                    psn = psN.tile([P, 512], F32, tag="psn", name="psn")
                    # full copy: zero rows double as finite init for the
                    # not-yet-computed head-pairs' factor rows
                    nq_mms(dkind, dlb, dms, dsqs, psn)
                    rows = LOW if len(dms) == 2 else slice(0, P)
                    emit_norms(dkind, slice(dlb * 512, (dlb + 1) * 512), rows)
                    for m in dms:
                        bc = psN.tile([P, 512], F32, tag="psn", name="bc")
                        emit_apply(dkind, dlb, m, bc)
                deferred = (kind, lb, ms, sqs) if kind is not None else None

        # ---------------- attention + deferred work ----------------
        zbp = stack.enter_context(tc.tile_pool(name="zbp", bufs=2))

        with tc.tile_pool(name="psS", bufs=2, space="PSUM") as psS, \
             tc.tile_pool(name="psOT", bufs=2, space="PSUM") as psOT, \
             tc.tile_pool(name="etp", bufs=4) as etp, \
             tc.tile_pool(name="zrp", bufs=2) as zrp, \
             tc.tile_pool(name="obp", bufs=2) as obp:

            def emit_epilogue(m, lq, ot0, ot1):
                """mt[m] = OT[0:64] * broadcast(1/Z) (ones-row), bf16 out."""
                sl = slice(lq * 512, (lq + 1) * 512)
                zr0 = zrp.tile([1, 512], F32R, tag="zr", name="zr0")
                zr1 = zrp.tile([1, 512], F32R, tag="zr", name="zr1")
                with nc.allow_low_precision(reason="f32r reciprocal of Z"):
                    nc.vector.reciprocal(out=zr0[:], in_=ot0[64:65, :])
                    nc.vector.reciprocal(out=zr1[:], in_=ot1[64:65, :])
                zbe = zbp.tile([64, 1024], F32R, tag="zbe", name="zbe")
                nc.gpsimd.partition_broadcast(zbe[:, 0:512], zr0[:], channels=64)
                nc.gpsimd.partition_broadcast(zbe[:, 512:1024], zr1[:], channels=64)
                nc.vector.tensor_tensor(out=mt[m][0:64, sl], in0=ot0[0:64, :],
                                        in1=zbe[:, 0:512], op=MULT)
                nc.vector.tensor_tensor(out=mt[m][64:128, sl], in0=ot1[0:64, :],
                                        in1=zbe[:, 512:1024], op=MULT)

            def emit_oproj_chunk(lq, mo):
                """One 128-row chunk of the out-projection for lq's block.
                Uses the PV accumulator pool so the score->exp pipeline's
                double buffer is never disturbed."""
                sl = slice(lq * 512, (lq + 1) * 512)
                pd = psOT.tile([P, 512], F32, tag=("ot0" if mo % 2 == 0 else "ot1"),
                               name="pd")
                for kc in range(NM):
                    nc.tensor.matmul(pd[:], lhsT=wot[:, kc, mo * P:(mo + 1) * P],
                                     rhs=mt[kc][:, sl],
                                     start=(kc == 0), stop=(kc == NM - 1))
                ob = obp.tile([P, 512], F32, tag="ob")
                nc.vector.tensor_copy(out=ob[:], in_=pd[:])
                nc.gpsimd.dma_start(
                    out=prm["out_t"][mo * P:(mo + 1) * P, sl], in_=ob[:])

            _att_slot = [0]

            def alloc_att():
                _att_slot[0] ^= 1
                return psOT.tile([P, 512], F32,
                                 tag=("ot0" if _att_slot[0] else "ot1"),
                                 name="qw")

            # Deferred projection work streamed into the attention groups:
            #  - K's head-pairs 2/3 (xk re-loaded; needed from group 2 on)
            #  - Q blocks 1..3 (needed by the matching lq groups)
            # Each stream batches its Ln/Exp pair over all its columns/rows
            # so the ACT table switch is paid once per stream.
            pending = []
            if "p" in phases and deferred is not None:
                # the last projection section's epilogue streams into the
                # first attention group (its applies are first read at lk12)
                # instead of serially gating the attention start
                dkind, dlb, dms, dsqs = deferred

                def last_epi_nq():
                    nq_mms(dkind, dlb, dms, dsqs, alloc_att())
                    emit_norms(dkind, slice(dlb * 512, (dlb + 1) * 512))
                pending.append(last_epi_nq)
                for m in dms:
                    pending.append(
                        lambda m=m: emit_apply(dkind, dlb, m, alloc_att()))
            if "p" in phases:
                pstate = {("q", lb): {} for lb in range(1, NLB)}
                # x loads self-pace on xp slot availability; issue the first
                # two up front so their transfers hide under the early groups
                pstate[("q", 1)]["x"] = load_x("xq", 1)
                pstate[("q", 2)]["x"] = load_x("xq", 2)

                def p_load(kind, lb):
                    pstate[(kind, lb)]["x"] = load_x("x" + kind, lb)

                def p_proj(kind, lb, m):
                    st = pstate[(kind, lb)]
                    pa = alloc_att()
                    proj_mms_m(kind, lb, m, st["x"][0], st["x"][1], pa)
                    st.setdefault("sqs", []).append(bias_sq_m(kind, lb, m, pa))

                def p_nq(kind, lb, ms, rows):
                    nq_mms(kind, lb, ms, pstate[(kind, lb)]["sqs"],
                           alloc_att(), rows)

                for lb in range(1, NLB):
                    for m in range(NM):
                        pending.append(lambda lb=lb, m=m: p_proj("q", lb, m))
                    pending.append(
                        lambda lb=lb: p_nq("q", lb, (0, 1, 2, 3),
                                           slice(0, P)))
                    if lb == 1:
                        pending.append(lambda: p_load("q", 3))
                pending.append(lambda: emit_norms("q", slice(512, L)))
                # spacer pops: give the ACT norm chain two exp-periods of
                # headroom so the apply matmuls never block the PE on it
                pending.append(lambda: None)
                pending.append(lambda: None)
                for lb in range(1, NLB):
                    for m in range(NM):
                        pending.append(
                            lambda lb=lb, m=m: emit_apply("q", lb, m,
                                                          alloc_att()))

            groups = [(lq, m) for lq in range(NLB) for m in range(NM)]
            if "c" not in phases:
                groups = []
            for lq, m in groups:
                ot0 = psOT.tile([65, 512], F32, tag="ot0")
                ot1 = psOT.tile([65, 512], F32, tag="ot1")
                for lk in range(NLK):
                    pss = psS.tile([P, 1024], F32, tag="pss", name="pss")
                    for s in range(2):
                        base = s * 64
                        nc.tensor.matmul(
                            pss[:, s * 512:(s + 1) * 512],
                            lhsT=kt[m][base:base + 64, lk * P:(lk + 1) * P],
                            rhs=qt[m][base:base + 64, lq * 512:(lq + 1) * 512],
                            start=True, stop=True)
                    # 7 pops per group: deadline analysis shows the streamed
                    # q blocks still land 1+ group before their lq needs
                    # them, and the gentler pace keeps the early groups
                    # ACT-bound instead of PE-bound.
                    if lk >= 5 and (lk % 2 == 1 or lk == 6) and pending:
                        pending.pop(0)()
                    et = etp.tile([P, 1024], BF16, tag="et")
                    nc.scalar.activation(out=et[:], in_=pss[:], func=EXP)
                    nc.tensor.matmul(ot0[:], lhsT=vg_all[:, lk, 2 * m, :],
                                     rhs=et[:, 0:512],
                                     start=(lk == 0), stop=(lk == NLK - 1),
                                     skip_group_check=True)
                    nc.tensor.matmul(ot1[:], lhsT=vg_all[:, lk, 2 * m + 1, :],
                                     rhs=et[:, 512:1024],
                                     start=(lk == 0), stop=(lk == NLK - 1),
                                     skip_group_check=True)
                emit_epilogue(m, lq, ot0, ot1)
                if m == NM - 1 and "d" in phases:
                    for mo in range(D // P):
                        pending.append(
                            lambda lq=lq, mo=mo: emit_oproj_chunk(lq, mo))
            # drain remaining deferred work
            for fn in pending:
                fn()
            if "c" not in phases:
                ob0 = obp.tile([P, 512], F32, tag="ob")
                nc.vector.memset(ob0[:], 0.0)
                nc.gpsimd.dma_start(out=prm["out_t"][0:P, 0:512], in_=ob0[:])


def build_nc(repeat=1, phases="pcd"):
    key = (repeat, phases)
    if key in _CACHE:
        return _CACHE[key]
    nc = bacc.Bacc("TRN2", target_bir_lowering=False, debug=False, num_devices=8)
    prm = {}
    for name in ("xq", "xk", "xv"):
        prm[name] = nc.declare_dram_parameter(name, [D, L], F32R, isOutput=False)
    for name in ("wq", "wk", "wv"):
        prm[name] = nc.declare_dram_parameter(name, [D, DO], F32R, isOutput=False)
    prm["wo"] = nc.declare_dram_parameter("wo", [DO, D], BF16, isOutput=False)
    prm["bqp"] = nc.declare_dram_parameter("bqp", [P, NM], F32, isOutput=False)
    prm["bkp"] = nc.declare_dram_parameter("bkp", [P, NM], F32, isOutput=False)
    prm["seln"] = nc.declare_dram_parameter("seln", [P, NM, P], F32R,
                                            isOutput=False)
    prm["selbc"] = nc.declare_dram_parameter("selbc", [P, NM, P], F32R,
                                             isOutput=False)
    prm["actc"] = nc.declare_dram_parameter("actc", [P, 2], F32, isOutput=False)
    prm["ones8"] = nc.declare_dram_parameter("ones8", [P, 8], BF16,
                                             isOutput=False)
    prm["out_t"] = nc.declare_dram_parameter("out_t", [D, L], F32, isOutput=True)
    _emit(nc, prm, repeat=repeat, phases=phases)
    nc.compile()
    _CACHE[key] = nc
    return nc


def make_in_maps(q, k, v, Wq, bq, Wk, bk, Wv, bv, Wo, bo):
    B = q.shape[0]
    f32 = np.float32
    bf16 = mybir.dt.np(BF16)

    selbc = np.zeros((P, NM, P), f32)
    for m in range(NM):
        for p in range(P):
            selbc[32 * m + (p // 64), m, p] = 1.0
    seln = np.zeros((P, NM, P), f32)
    for m in range(NM):
        for f in range(P):
            seln[f, m, 32 * m + (f // 64)] = 1.0
    actc = np.zeros((P, 2), f32)
    actc[:, 0] = 1e-24
    actc[:, 1] = LNTAUINV
    ones8 = np.ones((P, 8), f32).astype(bf16)

    in_maps = []
    for b in range(B):
        for g in range(2):
            sl = slice(g * DO, (g + 1) * DO)
            in_maps.append({
                "xq": np.ascontiguousarray(q[b].T.astype(f32)),
                "xk": np.ascontiguousarray(k[b].T.astype(f32)),
                "xv": np.ascontiguousarray(v[b].T.astype(f32)),
                "wq": np.ascontiguousarray(Wq[sl, :].T.astype(f32)),
                "wk": np.ascontiguousarray(Wk[sl, :].T.astype(f32)),
                "wv": np.ascontiguousarray(Wv[sl, :].T.astype(f32)),
                "wo": np.ascontiguousarray(Wo[:, sl].T.astype(f32)).astype(bf16),
                "bqp": np.ascontiguousarray(bq[sl].reshape(NM, P).T.astype(f32)),
                "bkp": np.ascontiguousarray(bk[sl].reshape(NM, P).T.astype(f32)),
                "seln": seln, "selbc": selbc, "actc": actc, "ones8": ones8,
            })
    return in_maps


def assemble(results, bv, Wo, bo):
    B = len(results) // 2
    bias = (bo + bv @ Wo.T).astype(np.float32)
    outs = []
    for b in range(B):
        part = (results[2 * b]["out_t"].astype(np.float32)
                + results[2 * b + 1]["out_t"].astype(np.float32))
        outs.append(part.T + bias)
    return np.stack(outs).astype(np.float32)


def kernel(q, k, v, Wq, bq, Wk, bk, Wv, bv, Wo, bo):
    q, k, v = (np.asarray(t, np.float32) for t in (q, k, v))
    Wq, bq, Wk, bk, Wv, bv, Wo, bo = (
        np.asarray(t, np.float32) for t in (Wq, bq, Wk, bk, Wv, bv, Wo, bo))
    nc = build_nc()
    in_maps = make_in_maps(q, k, v, Wq, bq, Wk, bk, Wv, bv, Wo, bo)
    last_err = None
    for attempt in range(3):
        try:
            res = run_bass_kernel_spmd(nc, in_maps, core_ids=list(range(8)))
            return assemble(res.results, bv, Wo, bo)
        except Exception as e:  # transient NRT device errors: retry
            last_err = e
            import time as _time
            _time.sleep(2.0)
    raise last_err


# revision 57
# speedup vs baseline: 1.1430x; 1.1430x over previous
"""Cosine cross-attention (B=4, L=2048, D=1024, H=16, dh=64, tau=0.07) on 8 trn2 cores.

Sharding: core = b*2 + g  (b in 0..3 data-parallel, g in 0..1 head-group of 8 heads).

Engine-overlap-oriented structure:
  prologue: V-proj, K-proj (per-block norms+applies software-pipelined one
  block behind the projections), Q block 0. Q blocks 1..3, the out-projection
  chunks, and their norm work stream INTO the attention loop as deferred
  items (one popped per lk iteration) so the ACT exp pipeline never waits.

  - One ACT table for everything: softmax exp plus the norm factors
    1/||x|| = exp(-0.5*ln(ss + eps)) (ln(1/tau) folded into k's exponent
    bias). The streamed q-norm Ln/Exp pair is batched over columns 512:2048
    so the Ln<->Exp table switch is paid once. Squares are done on DVE.
  - Norm row-sums: accumulated row-select matmuls (seln) write all 128 psum
    partitions (zeros elsewhere), so the broadcast source has no junk rows.
  - Z: DVE reciprocal of the PSUM ones-row, GpSimd partition_broadcast,
    fused multiply into mt during the PSUM->SBUF move. No DMA, no PE.
  - Attention is lq-outer / m-inner; out-projection chunks for lq run inside
    the (lq+1, m=0) group's exp stream via the deferred-work queue, using
    the PV-accumulator pool so the score double-buffer is untouched.
  - Bulk x loads are big SWDGE transfers on SP (first xv block leads the
    queue); weights + output stores are issued from GpSimd.
  - bf16 storage for qt/kt/vg/et/mt/wo; f32 PSUM accumulation everywhere.

Known remaining headroom (measured via the CoreSim cost model): the ~85us
prologue (V + K projections) runs with the ACT exp engine idle. The PE and
ACT totals are balanced (~292us each), so hiding the prologue requires
spreading its PE work across ALL attention groups' slack (~5us/group):
stream K's head-pairs 2/3 (norms via partition-halves of nsq, xk re-loaded
on the GpSimd queue) and the N=256 upper half of V. CRITICAL lesson from a
failed attempt: deferred items' emission order IS the dependency order —
a kt-apply emitted after a score matmul that reads those columns silently
computes with unnormalized K (rel err 0.47). Any such schedule needs
per-item deadline bookkeeping against the group/lk consumption schedule.
"""

import os

# some harnesses pin jax to cpu for the reference; this kernel needs the
# axon/neuron backend, so clear the pin before jax is first imported
if os.environ.get("JAX_PLATFORMS") == "cpu":
    del os.environ["JAX_PLATFORMS"]

import math

import numpy as np

import concourse.bacc as bacc
import concourse.tile as tile
from concourse import mybir
from concourse.bass_utils import run_bass_kernel_spmd

P = 128
L = 2048
D = 1024
DO = 512  # per-core output dims of q/k/v projections (8 heads * 64)
TAU = 0.07
NLB = L // 512   # 4 blocks of 512 along L
NLK = L // 128   # 16 chunks of 128 along L (keys)
NM = DO // P     # 4 dout chunks (head pairs)
NKC = D // P     # 8 contraction chunks for projections

F32 = mybir.dt.float32
F32R = mybir.dt.float32r
BF16 = mybir.dt.bfloat16
EXP = mybir.ActivationFunctionType.Exp
LN = mybir.ActivationFunctionType.Ln
MULT = mybir.AluOpType.mult
LNTAUINV = -math.log(TAU)  # fold 1/tau into k's norm factor

_CACHE = {}


def _emit(nc, prm, repeat=1, phases="pcd"):
    with tile.TileContext(nc) as tc:
        if repeat > 1:
            with tc.For_i(0, repeat, 1):
                _emit_body(nc, tc, prm, phases)
        else:
            _emit_body(nc, tc, prm, phases)


def _emit_body(nc, tc, prm, phases="pcd"):
    from contextlib import ExitStack
    with ExitStack() as stack:
        const = stack.enter_context(tc.tile_pool(name="const", bufs=1))
        persist = stack.enter_context(tc.tile_pool(name="persist", bufs=1))

        # ---- first x block loads lead the SP queue (v-proj gates on them) ----
        xp = stack.enter_context(tc.tile_pool(name="xp", bufs=3))

        def load_x(name, lb, eng=None):
            eng = eng or nc.sync
            sl = slice(lb * 512, (lb + 1) * 512)
            x0 = xp.tile([P, NKC // 2, 512], F32R, tag="x", name="x0")
            eng.dma_start(
                out=x0[:],
                in_=prm[name][0:512, sl].rearrange("(c p) i -> p c i", c=4))
            x1 = xp.tile([P, NKC // 2, 512], F32R, tag="x", name="x1")
            eng.dma_start(
                out=x1[:],
                in_=prm[name][512:1024, sl].rearrange("(c p) i -> p c i", c=4))
            return x0, x1

        xv_first = load_x("xv", 0)

        # ---- constants ----
        seln = const.tile([P, NM, P], F32R, tag="seln")
        nc.sync.dma_start(out=seln[:], in_=prm["seln"][:])
        selbc = const.tile([P, NM, P], F32R, tag="selbc")
        nc.sync.dma_start(out=selbc[:], in_=prm["selbc"][:])
        bq_t = const.tile([P, NM], F32, tag="bq")
        nc.sync.dma_start(out=bq_t[:], in_=prm["bqp"][:])
        bk_t = const.tile([P, NM], F32, tag="bk")
        nc.sync.dma_start(out=bk_t[:], in_=prm["bkp"][:])
        # col 0: ln-eps bias, col 1: ln(1/tau) (k's exponent bias)
        actc = const.tile([P, 2], F32, tag="actc")
        nc.sync.dma_start(out=actc[:], in_=prm["actc"][:])
        ones8 = const.tile([P, 8], BF16, tag="ones8")
        nc.sync.dma_start(out=ones8[:], in_=prm["ones8"][:])

        # ---- persistent tensors ----
        qt = [persist.tile([P, L], BF16, tag=f"qt{m}", name=f"qt{m}") for m in range(NM)]
        kt = [persist.tile([P, L], BF16, tag=f"kt{m}", name=f"kt{m}") for m in range(NM)]
        mt = [persist.tile([P, L], BF16, tag=f"mt{m}", name=f"mt{m}") for m in range(NM)]
        vg_all = persist.tile([P, NLK, 8, 65], BF16, tag="vg_all")
        nsq = {"q": persist.tile([P, L], F32R, tag="nsq_q", name="nsq_q"),
               "k": persist.tile([P, L], F32R, tag="nsq_k", name="nsq_k")}
        wot = persist.tile([P, NM, D], BF16, tag="wot")

        wp = stack.enter_context(tc.tile_pool(name="wp", bufs=2))
        sqp = stack.enter_context(tc.tile_pool(name="sqp", bufs=8))

        def load_w(name):
            w_t = wp.tile([P, NKC, DO], F32R, tag="w", name=f"w_{name}")
            nc.gpsimd.dma_start(
                out=w_t[:],
                in_=prm[name][:].rearrange("(c p) i -> p c i", c=NKC))
            return w_t

        wv_t = load_w("wv")
        wk_t = load_w("wk")

        # ---------------- V projection (natural layout) ----------------
        with tc.tile_pool(name="psV", bufs=2, space="PSUM") as psV:
            for lb in range(NLB if "p" in phases else 0):
                x0, x1 = xv_first if lb == 0 else load_x("xv", lb)
                for j in range(4):
                    pav = psV.tile([P, 512], F32, tag="pav")
                    for c8 in range(NKC):
                        xt = (x0 if c8 < 4 else x1)
                        nc.tensor.matmul(
                            pav[:],
                            lhsT=xt[:, c8 % 4, j * P:(j + 1) * P],
                            rhs=wv_t[:, c8, :],
                            start=(c8 == 0), stop=(c8 == NKC - 1))
                    lc = lb * 4 + j
                    nc.vector.tensor_copy(
                        out=vg_all[:, lc, :, 0:64],
                        in_=pav[:].rearrange("p (h d) -> p h d", h=8))
                    nc.vector.tensor_copy(out=vg_all[:, lc, :, 64],
                                          in_=ones8[:])
        wq_t = load_w("wq")  # reuses wv's slot; hidden behind k-projection
        nc.gpsimd.dma_start(
            out=wot[:],
            in_=prm["wo"][:].rearrange("(c p) i -> p c i", c=NM))

        # ---------------- Q/K projections + norm factors ----------------
        # Emission primitives shared by the pre-attention path (dedicated
        # psum banks, kc-outer, epilogues deferred one block so the PE never
        # waits on a DVE round trip) and the streamed-q path (small items
        # borrowing PV-pool slots inside the attention loop).
        def proj_mms_m(kind, lb, m, x0, x1, pa):
            w_t = wk_t if kind == "k" else wq_t
            for c8 in range(NKC):
                xt = (x0 if c8 < 4 else x1)
                nc.tensor.matmul(
                    pa[:], lhsT=w_t[:, c8, m * P:(m + 1) * P],
                    rhs=xt[:, c8 % 4, :],
                    start=(c8 == 0), stop=(c8 == NKC - 1))

        def bias_sq_m(kind, lb, m, pa):
            """bias-add into qt/kt (bf16/f32r) + squares; returns sq tile."""
            b_t = bq_t if kind == "q" else bk_t
            dst = qt if kind == "q" else kt
            blk = dst[m][:, slice(lb * 512, (lb + 1) * 512)]
            nc.vector.tensor_scalar_add(out=blk, in0=pa[:],
                                        scalar1=b_t[:, m:m + 1])
            sq_t = sqp.tile([P, 512], F32R, tag="sq")
            nc.vector.tensor_tensor(out=sq_t[:], in0=blk, in1=blk, op=MULT)
            return sq_t

        def nq_mms(kind, lb, ms, sqs, psn, rows=slice(0, P)):
            """Head-pair square-sums: accumulated row-select matmuls write ALL
            128 partitions of psn (zeros elsewhere -> no junk rows); only the
            `rows` half is copied into nsq (the other half may hold factors
            from an earlier pass)."""
            sl = slice(lb * 512, (lb + 1) * 512)
            for i, (m, sq_t) in enumerate(zip(ms, sqs)):
                nc.tensor.matmul(psn[:], lhsT=seln[:, m, :], rhs=sq_t[:],
                                 start=(i == 0), stop=(i == len(ms) - 1))
            nc.vector.tensor_copy(out=nsq[kind][rows, sl], in_=psn[rows, :])

        def emit_norms(kind, cols, rows=slice(0, P)):
            # 1/||x|| = exp(-0.5*ln(ss + eps)); ln(1/tau) folded into k's bias
            nb = nsq[kind][rows, cols]
            with nc.allow_low_precision(reason="norms via ln/exp"):
                nc.scalar.activation(out=nb, in_=nb, func=LN,
                                     bias=actc[rows, 0:1])
                if kind == "q":
                    nc.scalar.activation(out=nb, in_=nb, func=EXP, scale=-0.5)
                else:
                    nc.scalar.activation(out=nb, in_=nb, func=EXP, scale=-0.5,
                                         bias=actc[rows, 1:2])

        def emit_apply(kind, lb, m, bc):
            sl = slice(lb * 512, (lb + 1) * 512)
            nc.tensor.matmul(bc[:], lhsT=selbc[:, m, :],
                             rhs=nsq[kind][:, sl], start=True, stop=True)
            blk = (qt if kind == "q" else kt)[m][:, sl]
            nc.vector.tensor_tensor(out=blk, in0=blk, in1=bc[:], op=MULT)

        # pre-attention prologue: K's first two head-pairs (what the first
        # two attention groups read) + Q's first block. K's other head-pairs
        # and Q's other blocks stream into the attention loop as deferred
        # work. Software-pipelined: block i's norm/apply matmuls run behind
        # block i+1's projections so the PE never waits on a DVE round trip.
        LOW, HIGH = slice(0, 64), slice(64, P)
        with tc.tile_pool(name="psA", bufs=1, space="PSUM") as psA, \
             tc.tile_pool(name="psN", bufs=2, space="PSUM") as psN:
            sections = ([("k", lb, (0, 1, 2, 3)) for lb in range(NLB)]
                        + [("q", 0, (0, 1, 2, 3))] if "p" in phases else [])
            deferred = None
            for kind, lb, ms in sections + [(None, None, None)]:
                if kind is not None:
                    x0, x1 = load_x("x" + kind, lb)
                    pas = [psA.tile([P, 512], F32, tag=f"pa{m}", name="pa")
                           for m in ms]
                    for c8 in range(NKC):
                        xt = (x0 if c8 < 4 else x1)
                        for i, m in enumerate(ms):
                            nc.tensor.matmul(
                                pas[i][:],
                                lhsT=(wk_t if kind == "k" else wq_t)[
                                    :, c8, m * P:(m + 1) * P],
                                rhs=xt[:, c8 % 4, :],
                                start=(c8 == 0), stop=(c8 == NKC - 1))
                    sqs = [bias_sq_m(kind, lb, m, pas[i])
                           for i, m in enumerate(ms)]
                if deferred is not None:
                    dkind, dlb, dms, dsqs = deferred
                    psn = psN.tile([P, 512], F32, tag="psn", name="psn")
                    # full copy: zero rows double as finite init for the
                    # not-yet-computed head-pairs' factor rows
                    nq_mms(dkind, dlb, dms, dsqs, psn)
                    rows = LOW if len(dms) == 2 else slice(0, P)
                    emit_norms(dkind, slice(dlb * 512, (dlb + 1) * 512), rows)
                    for m in dms:
                        bc = psN.tile([P, 512], F32, tag="psn", name="bc")
                        emit_apply(dkind, dlb, m, bc)
                deferred = (kind, lb, ms, sqs) if kind is not None else None

        # ---------------- attention + deferred work ----------------
        zbp = stack.enter_context(tc.tile_pool(name="zbp", bufs=2))

        with tc.tile_pool(name="psS", bufs=2, space="PSUM") as psS, \
             tc.tile_pool(name="psOT", bufs=2, space="PSUM") as psOT, \
             tc.tile_pool(name="etp", bufs=4) as etp, \
             tc.tile_pool(name="zrp", bufs=2) as zrp, \
             tc.tile_pool(name="obp", bufs=2) as obp:

            def emit_epilogue(m, lq, ot0, ot1):
                """mt[m] = OT[0:64] * broadcast(1/Z) (ones-row), bf16 out."""
                sl = slice(lq * 512, (lq + 1) * 512)
                zr0 = zrp.tile([1, 512], F32R, tag="zr", name="zr0")
                zr1 = zrp.tile([1, 512], F32R, tag="zr", name="zr1")
                with nc.allow_low_precision(reason="f32r reciprocal of Z"):
                    nc.vector.reciprocal(out=zr0[:], in_=ot0[64:65, :])
                    nc.vector.reciprocal(out=zr1[:], in_=ot1[64:65, :])
                zbe = zbp.tile([64, 1024], F32R, tag="zbe", name="zbe")
                nc.gpsimd.partition_broadcast(zbe[:, 0:512], zr0[:], channels=64)
                nc.gpsimd.partition_broadcast(zbe[:, 512:1024], zr1[:], channels=64)
                nc.vector.tensor_tensor(out=mt[m][0:64, sl], in0=ot0[0:64, :],
                                        in1=zbe[:, 0:512], op=MULT)
                nc.vector.tensor_tensor(out=mt[m][64:128, sl], in0=ot1[0:64, :],
                                        in1=zbe[:, 512:1024], op=MULT)

            def emit_oproj_chunk(lq, mo):
                """One 128-row chunk of the out-projection for lq's block.
                Uses the PV accumulator pool so the score->exp pipeline's
                double buffer is never disturbed."""
                sl = slice(lq * 512, (lq + 1) * 512)
                pd = psOT.tile([P, 512], F32, tag=("ot0" if mo % 2 == 0 else "ot1"),
                               name="pd")
                for kc in range(NM):
                    nc.tensor.matmul(pd[:], lhsT=wot[:, kc, mo * P:(mo + 1) * P],
                                     rhs=mt[kc][:, sl],
                                     start=(kc == 0), stop=(kc == NM - 1))
                ob = obp.tile([P, 512], F32, tag="ob")
                nc.vector.tensor_copy(out=ob[:], in_=pd[:])
                nc.gpsimd.dma_start(
                    out=prm["out_t"][mo * P:(mo + 1) * P, sl], in_=ob[:])

            _att_slot = [0]

            def alloc_att():
                _att_slot[0] ^= 1
                return psOT.tile([P, 512], F32,
                                 tag=("ot0" if _att_slot[0] else "ot1"),
                                 name="qw")

            # Deferred projection work streamed into the attention groups:
            #  - K's head-pairs 2/3 (xk re-loaded; needed from group 2 on)
            #  - Q blocks 1..3 (needed by the matching lq groups)
            # Each stream batches its Ln/Exp pair over all its columns/rows
            # so the ACT table switch is paid once per stream.
            pending = []
            if "p" in phases and deferred is not None:
                # the last projection section's epilogue streams into the
                # first attention group (its applies are first read at lk12)
                # instead of serially gating the attention start
                dkind, dlb, dms, dsqs = deferred

                def last_epi_nq():
                    nq_mms(dkind, dlb, dms, dsqs, alloc_att())
                    emit_norms(dkind, slice(dlb * 512, (dlb + 1) * 512))
                pending.append(last_epi_nq)
                for m in dms:
                    pending.append(
                        lambda m=m: emit_apply(dkind, dlb, m, alloc_att()))
            if "p" in phases:
                pstate = {("q", lb): {} for lb in range(1, NLB)}
                # x loads self-pace on xp slot availability; issue the first
                # two up front so their transfers hide under the early groups
                pstate[("q", 1)]["x"] = load_x("xq", 1)
                pstate[("q", 2)]["x"] = load_x("xq", 2)

                def p_load(kind, lb):
                    pstate[(kind, lb)]["x"] = load_x("x" + kind, lb)

                def p_proj(kind, lb, m):
                    st = pstate[(kind, lb)]
                    pa = alloc_att()
                    proj_mms_m(kind, lb, m, st["x"][0], st["x"][1], pa)
                    st.setdefault("sqs", []).append(bias_sq_m(kind, lb, m, pa))

                def p_nq(kind, lb, ms, rows):
                    nq_mms(kind, lb, ms, pstate[(kind, lb)]["sqs"],
                           alloc_att(), rows)

                for lb in range(1, NLB):
                    for m in range(NM):
                        pending.append(lambda lb=lb, m=m: p_proj("q", lb, m))
                    pending.append(
                        lambda lb=lb: p_nq("q", lb, (0, 1, 2, 3),
                                           slice(0, P)))
                    if lb == 1:
                        pending.append(lambda: p_load("q", 3))
                pending.append(lambda: emit_norms("q", slice(512, L)))
                # spacer pops: give the ACT norm chain two exp-periods of
                # headroom so the apply matmuls never block the PE on it
                pending.append(lambda: None)
                pending.append(lambda: None)
                for lb in range(1, NLB):
                    for m in range(NM):
                        pending.append(
                            lambda lb=lb, m=m: emit_apply("q", lb, m,
                                                          alloc_att()))

            groups = [(lq, m) for lq in range(NLB) for m in range(NM)]
            if "c" not in phases:
                groups = []
            for lq, m in groups:
                ot0 = psOT.tile([65, 512], F32, tag="ot0")
                ot1 = psOT.tile([65, 512], F32, tag="ot1")
                for lk in range(NLK):
                    pss = psS.tile([P, 1024], F32, tag="pss", name="pss")
                    for s in range(2):
                        base = s * 64
                        nc.tensor.matmul(
                            pss[:, s * 512:(s + 1) * 512],
                            lhsT=kt[m][base:base + 64, lk * P:(lk + 1) * P],
                            rhs=qt[m][base:base + 64, lq * 512:(lq + 1) * 512],
                            start=True, stop=True)
                    # 7 pops per group: deadline analysis shows the streamed
                    # q blocks still land 1+ group before their lq needs
                    # them, and the gentler pace keeps the early groups
                    # ACT-bound instead of PE-bound.
                    if lk >= 5 and (lk % 2 == 1 or lk == 6) and pending:
                        pending.pop(0)()
                    et = etp.tile([P, 1024], BF16, tag="et")
                    nc.scalar.activation(out=et[:], in_=pss[:], func=EXP)
                    nc.tensor.matmul(ot0[:], lhsT=vg_all[:, lk, 2 * m, :],
                                     rhs=et[:, 0:512],
                                     start=(lk == 0), stop=(lk == NLK - 1),
                                     skip_group_check=True)
                    nc.tensor.matmul(ot1[:], lhsT=vg_all[:, lk, 2 * m + 1, :],
                                     rhs=et[:, 512:1024],
                                     start=(lk == 0), stop=(lk == NLK - 1),
                                     skip_group_check=True)
                emit_epilogue(m, lq, ot0, ot1)
                if m == NM - 1 and "d" in phases:
                    for mo in range(D // P):
                        pending.append(
                            lambda lq=lq, mo=mo: emit_oproj_chunk(lq, mo))
            # drain remaining deferred work
            for fn in pending:
                fn()
            if "c" not in phases:
                ob0 = obp.tile([P, 512], F32, tag="ob")
                nc.vector.memset(ob0[:], 0.0)
                nc.gpsimd.dma_start(out=prm["out_t"][0:P, 0:512], in_=ob0[:])


def build_nc(repeat=1, phases="pcd"):
    key = (repeat, phases)
    if key in _CACHE:
        return _CACHE[key]
    nc = bacc.Bacc("TRN2", target_bir_lowering=False, debug=False, num_devices=8)
    prm = {}
    for name in ("xq", "xk", "xv"):
        prm[name] = nc.declare_dram_parameter(name, [D, L], F32R, isOutput=False)
    for name in ("wq", "wk", "wv"):
        prm[name] = nc.declare_dram_parameter(name, [D, DO], F32R, isOutput=False)
    prm["wo"] = nc.declare_dram_parameter("wo", [DO, D], BF16, isOutput=False)
    prm["bqp"] = nc.declare_dram_parameter("bqp", [P, NM], F32, isOutput=False)
    prm["bkp"] = nc.declare_dram_parameter("bkp", [P, NM], F32, isOutput=False)
    prm["seln"] = nc.declare_dram_parameter("seln", [P, NM, P], F32R,
                                            isOutput=False)
    prm["selbc"] = nc.declare_dram_parameter("selbc", [P, NM, P], F32R,
                                             isOutput=False)
    prm["actc"] = nc.declare_dram_parameter("actc", [P, 2], F32, isOutput=False)
    prm["ones8"] = nc.declare_dram_parameter("ones8", [P, 8], BF16,
                                             isOutput=False)
    prm["out_t"] = nc.declare_dram_parameter("out_t", [D, L], F32, isOutput=True)
    _emit(nc, prm, repeat=repeat, phases=phases)
    nc.compile()
    _CACHE[key] = nc
    return nc


def make_in_maps(q, k, v, Wq, bq, Wk, bk, Wv, bv, Wo, bo):
    B = q.shape[0]
    f32 = np.float32
    bf16 = mybir.dt.np(BF16)

    selbc = np.zeros((P, NM, P), f32)
    for m in range(NM):
        for p in range(P):
            selbc[32 * m + (p // 64), m, p] = 1.0
    seln = np.zeros((P, NM, P), f32)
    for m in range(NM):
        for f in range(P):
            seln[f, m, 32 * m + (f // 64)] = 1.0
    actc = np.zeros((P, 2), f32)
    actc[:, 0] = 1e-24
    actc[:, 1] = LNTAUINV
    ones8 = np.ones((P, 8), f32).astype(bf16)

    in_maps = []
    for b in range(B):
        for g in range(2):
            sl = slice(g * DO, (g + 1) * DO)
            in_maps.append({
                "xq": np.ascontiguousarray(q[b].T.astype(f32)),
                "xk": np.ascontiguousarray(k[b].T.astype(f32)),
                "xv": np.ascontiguousarray(v[b].T.astype(f32)),
                "wq": np.ascontiguousarray(Wq[sl, :].T.astype(f32)),
                "wk": np.ascontiguousarray(Wk[sl, :].T.astype(f32)),
                "wv": np.ascontiguousarray(Wv[sl, :].T.astype(f32)),
                "wo": np.ascontiguousarray(Wo[:, sl].T.astype(f32)).astype(bf16),
                "bqp": np.ascontiguousarray(bq[sl].reshape(NM, P).T.astype(f32)),
                "bkp": np.ascontiguousarray(bk[sl].reshape(NM, P).T.astype(f32)),
                "seln": seln, "selbc": selbc, "actc": actc, "ones8": ones8,
            })
    return in_maps


def assemble(results, bv, Wo, bo):
    B = len(results) // 2
    bias = (bo + bv @ Wo.T).astype(np.float32)
    outs = []
    for b in range(B):
        part = (results[2 * b]["out_t"].astype(np.float32)
                + results[2 * b + 1]["out_t"].astype(np.float32))
        outs.append(part.T + bias)
    return np.stack(outs).astype(np.float32)


def kernel(q, k, v, Wq, bq, Wk, bk, Wv, bv, Wo, bo):
    q, k, v = (np.asarray(t, np.float32) for t in (q, k, v))
    Wq, bq, Wk, bk, Wv, bv, Wo, bo = (
        np.asarray(t, np.float32) for t in (Wq, bq, Wk, bk, Wv, bv, Wo, bo))
    nc = build_nc()
    in_maps = make_in_maps(q, k, v, Wq, bq, Wk, bk, Wv, bv, Wo, bo)
    last_err = None
    for attempt in range(3):
        try:
            res = run_bass_kernel_spmd(nc, in_maps, core_ids=list(range(8)))
            return assemble(res.results, bv, Wo, bo)
        except Exception as e:  # transient NRT device errors: retry
            last_err = e
            import time as _time
            _time.sleep(2.0)
    raise last_err
